# revision 37
# baseline (speedup 1.0000x reference)
"""Causal multi-head self-attention (B=8, S=2048, D=384, H=4, Hd=96) on 8
Trainium2 NeuronCores.

Sharding: data-parallel over batch — each core processes one batch element,
weights replicated. No collectives needed.

Per-core algorithm (flash-style, fully SBUF-resident, no attention matrix in
HBM); everything feeding the PE is bf16 (1 cycle/col at any moving width,
halves DMA + SBUF so all cross-iteration pools double-buffer):
  - host passes x[b] pre-transposed as xT [384, 2048] in bf16 (layout prep
    and dtype cast only; all FLOPs run on device)
  - QT/KT per head in [96, S] layout (d on partitions), cast bf16 during the
    PSUM->SBUF drain (DVE). K-bias dropped: softmax over k is invariant to
    per-q constants and bk only contributes bk.(Q+bq), constant in k. Q-bias
    rides a rank-1 (bq-row x ones) matmul in the same PSUM accumulation.
  - V' = [ones | V_h] natural layout [S, 97*4] via augmented weights (bias +
    ones column folded into the contraction); ones column FIRST so the
    softmax denominator lands on PSUM partition 0.
  - projections are emitted interleaved with the previous attention chunk
    (own 1-bank PSUM pool) so the scheduler slots their matmuls into
    attention-phase PE gaps and the Activation engine never drains between
    chunks; DMAs split across both HWDGE queues, x first.
  - scoresT[k, q] = KT_h^T @ QT_h per 128-row k-tile and 512-col q-chunk,
    computing only cols >= rt for diagonal tiles; exp on ScalarE
    (PSUM->SBUF bf16, scale=1/sqrt(Hd) folded in, ranges trimmed to the
    computed region); both causal diagonal blocks of a group zeroed by ONE
    strided-AP mask multiply on DVE
  - OT' accumulated in PSUM: row 0 = softmax denominator, rows 1..96 =
    unnormalized head output (d x q)
  - reciprocal (custom DVE approx, reads PSUM row 0 directly) ->
    partition_broadcast (Pool) -> normalize rows 0..96 on DVE; row 0 becomes
    den * (1/den) = 1.0, the ones-row that folds the output bias into head
    0's output-projection matmul (97-partition contraction against
    wo rows [bo ; Wo_h]).
  - output projection summed in PSUM across heads, drained on DVE, DMA out
"""

import os
import sys

sys.path.insert(0, "/opt/trn_rl_repo")

import numpy as np

import concourse.bass as bass
import concourse.tile as tile
from concourse import bacc, mybir
from concourse.bass_utils import run_bass_kernel_spmd

N_CORES = 8
S = 2048
D = 384
H = 4
HD = 96
CH = 512          # q-chunk width (columns per matmul)
NCH = S // CH     # 4 q-chunks
P = 128           # k-tile height / partition dim
KTN = S // P      # 16 k-tiles
SCALE = 1.0 / np.sqrt(HD)

F32 = mybir.dt.float32
BF16 = mybir.dt.bfloat16
MM_DT = os.environ.get("ATTN_MM_DT", "float32r")  # float32r | float32
AT_MODE = os.environ.get("ATTN_AT", "bf16")       # bf16 | f32 attention core


def build_nc(repeat=1, variant=(), loop_n=0, qbias=False):
    """qbias: emit the rank-1 Q-bias matmuls. kernel() passes True only
    when bq is actually nonzero (host_prep sees the values), so the zero-bias
    case skips 16 matmuls without losing generality."""
    nc = bacc.Bacc("TRN2", target_bir_lowering=False, debug=False,
                   enable_asserts=False, num_devices=N_CORES)
    # MF: dtype for tensors feeding matmuls (float32r = single-pass relaxed
    # fp32 on the PE, 4x faster than true fp32; same 4-byte numpy layout).
    # AT: attention-core dtype (bf16: 1 cycle/col at any moving width).
    MF = mybir.dt.float32r if MM_DT == "float32r" else F32
    AT = BF16 if AT_MODE == "bf16" and "at_f32" not in variant else MF

    xt_d = nc.dram_tensor("xt", [D, S], AT, kind="ExternalInput").ap()
    wq_d = nc.dram_tensor("wq", [D, D], AT, kind="ExternalInput").ap()
    wk_d = nc.dram_tensor("wk", [D, D], AT, kind="ExternalInput").ap()
    wvx_d = nc.dram_tensor("wvx", [D + 1, 97 * H], AT, kind="ExternalInput").ap()
    woa_d = nc.dram_tensor("woa", [97 * H, D], AT, kind="ExternalInput").ap()
    bqr_d = (nc.dram_tensor("bqr", [1, D], AT, kind="ExternalInput").ap()
             if qbias else None)
    msk_d = nc.dram_tensor("msk", [P, P], AT, kind="ExternalInput").ap()
    ones_d = nc.dram_tensor("onesrow", [1, S], AT, kind="ExternalInput").ap()
    out_d = nc.dram_tensor("out", [S, D], F32, kind="ExternalOutput").ap()

    Exp = mybir.ActivationFunctionType.Exp
    mult = mybir.AluOpType.mult

    with tile.TileContext(nc) as tc:
        wpool = tc.alloc_tile_pool(name="w", bufs=1)
        xpool = tc.alloc_tile_pool(name="x", bufs=2)
        qkt_pool = tc.alloc_tile_pool(name="qkt", bufs=2)
        vpool = tc.alloc_tile_pool(name="v", bufs=2)
        ppool = tc.alloc_tile_pool(name="p", bufs=(8 if "p8" in variant
                                                    else 4))
        DP3 = 2 if "deep2" in variant else 3
        onpool = tc.alloc_tile_pool(name="on", bufs=DP3)
        rpool = tc.alloc_tile_pool(name="r", bufs=DP3)
        GRP = 1 if "g1" in variant else 2
        HP = "hp" in variant
        A4 = "a4" in variant
        qkpool = tc.alloc_tile_pool(
            name="qkps", bufs=(4 if GRP == 1 else (3 if HP else 2)),
            space="PSUM")
        accpool = tc.alloc_tile_pool(
            name="accps", bufs=(2 if HP else (4 if A4 else 3)), space="PSUM")
        projpool = ((qkpool if HP or A4 else
                     tc.alloc_tile_pool(name="projps", bufs=1, space="PSUM")))

        import contextlib
        loop_ctx = (tc.For_i(0, loop_n, 1) if loop_n
                    else contextlib.nullcontext())
        with loop_ctx:
          for _rep in range(repeat):
              # ---- load inputs / weights. xt on the SP queue first (it
              # gates the first projections); weights go on the Activation
              # HWDGE queue so both queues fill in parallel.
              wdma = nc.scalar.dma_start
              xt_sb, wq_sb, wk_sb, wv_sb, wo_sb = [], [], [], [], []
              for t in range(3):
                  xt = xpool.tile([P, S], AT, name=f"xt{t}", tag=f"xt{t}")
                  (nc.sync.dma_start if t < 2 else wdma)(
                      xt[:], xt_d[P * t:P * t + P, :])
                  xt_sb.append(xt)
              for t in range(3):
                  wkt = wpool.tile([P, D], AT, name=f"wk{t}", tag=f"wk{t}")
                  wdma(wkt[:], wk_d[P * t:P * t + P, :])
                  wk_sb.append(wkt)
              for t in range(3):
                  wqt = wpool.tile([P, D], AT, name=f"wq{t}", tag=f"wq{t}")
                  wdma(wqt[:], wq_d[P * t:P * t + P, :])
                  wq_sb.append(wqt)
              if qbias:
                  bqr_sb = wpool.tile([1, D], AT, name="bqr", tag="bqr")
                  wdma(bqr_sb[:], bqr_d[:, :])
              msk_sb = wpool.tile([P, P], AT, name="msk", tag="msk")
              wdma(msk_sb[:], msk_d[:, :])
              ones = wpool.tile([1, S], AT, name="ones", tag="ones")
              wdma(ones[:], ones_d[:, :])
              for t in range(3):
                  wvt = wpool.tile([P, 97 * H], AT, name=f"wv{t}", tag=f"wv{t}")
                  wdma(wvt[:], wvx_d[P * t:P * t + P, :])
                  wv_sb.append(wvt)
              wvb = wpool.tile([1, 97 * H], AT, name="wvb", tag="wvb")
              wdma(wvb[:], wvx_d[D:D + 1, :])
              for h in range(H):
                  wot = wpool.tile([97, D], AT, name=f"wo{h}", tag=f"wo{h}")
                  wdma(wot[:], woa_d[97 * h:97 * h + 97, :])
                  wo_sb.append(wot)

              # ---- allocate Q/K/V destination tiles ----
              qt_sb, kt_sb = [], []
              for h in range(H):
                  qt = qkt_pool.tile([HD, S], AT, name=f"qt{h}", tag=f"qt{h}")
                  qt_sb.append(qt)
                  kt = qkt_pool.tile([HD, S], AT, name=f"kt{h}", tag=f"kt{h}")
                  kt_sb.append(kt)
              v_sb = [None] * KTN

              cp = (nc.scalar.copy if "actcopy" in variant
                    else nc.vector.tensor_copy)

              def proj_items(kc, two_pools=False):
                  """Projection work feeding attention chunk kc, as emit
                  thunks: (K,Q) per head first — the next chunk's QK matmuls
                  unblock per head — with V row-tiles interleaved early for
                  the first PV groups. Own 1-bank PSUM pool so these slot
                  into attention-phase PE gaps without blocking score/acc
                  tiles; batch 0 (nothing to overlap with) also borrows the
                  idle score pool for 3-deep pipelining."""
                  seq = []
                  for h in range(H):
                      seq.append(("k", h))
                      seq.append(("q", h))
                      if h < 2:
                          seq.append(("v", 4 * kc + 2 * h))
                          seq.append(("v", 4 * kc + 2 * h + 1))
                  for item_i, (kind, idx) in enumerate(seq):
                      use_qk = HP or A4 or (two_pools and item_i % 3 == 2)
                      def emit(kind=kind, idx=idx, use_qk=use_qk):
                          if use_qk:
                              ps = qkpool.tile([P, GRP * CH], F32, name="pps",
                                               tag="qk")
                          else:
                              ps = projpool.tile([P, CH], F32, name="pps",
                                                 tag="pps")
                          if kind == "v":
                              st = idx
                              for t in range(3):
                                  nc.tensor.matmul(
                                      ps[:, 0:97 * H],
                                      xt_sb[t][:, P * st:P * st + P],
                                      wv_sb[t][:], start=(t == 0), stop=False)
                              nc.tensor.matmul(ps[:, 0:97 * H], ones[:, 0:P],
                                               wvb[:], start=False, stop=True)
                              vt = vpool.tile([P, 97 * H], AT, name=f"v{st}",
                                              tag=f"v{st}")
                              cp(vt[:], ps[:, 0:97 * H])
                              v_sb[st] = vt
                              return
                          h = idx
                          w_sb, dst, has_bias = (
                              (wk_sb, kt_sb, False) if kind == "k"
                              else (wq_sb, qt_sb, qbias))
                          for t in range(3):
                              nc.tensor.matmul(
                                  ps[0:HD, 0:CH],
                                  w_sb[t][:, HD * h:HD * h + HD],
                                  xt_sb[t][:, CH * kc:CH * kc + CH],
                                  start=(t == 0),
                                  stop=(t == 2 and not has_bias))
                          if has_bias:
                              nc.tensor.matmul(
                                  ps[0:HD, 0:CH],
                                  bqr_sb[:, HD * h:HD * h + HD],
                                  ones[:, 0:CH], start=False, stop=True)
                          if "nocopy" in variant:
                              d8 = dst[h][:, CH * kc:CH * kc + 64].copy()
                              d8.ap = d8.ap[:1] + [[64, CH // 64], [1, 8]]
                              s8 = ps[0:HD, 0:64].copy()
                              s8.ap = s8.ap[:1] + [[64, CH // 64], [1, 8]]
                              cp(d8, s8)
                          else:
                              cp(dst[h][:, CH * kc:CH * kc + CH],
                                 ps[0:HD, 0:CH])
                      yield emit

              # ---- attention; chunk ci+1's projections interleave into
              # ---- chunk ci's group loop (one item per group iteration)
              for it in proj_items(0, two_pools=True):
                  it()
              pending = []
              for ci in range(NCH):
                  if ci + 1 < NCH:
                      pending = list(proj_items(ci + 1))
                  n_iters = H * (4 * (ci + 1) // GRP)
                  per_iter = -(-len(pending) // n_iters) if pending else 0
                  on_tiles = []
                  for h in range(H):
                      nkt = 4 * (ci + 1)
                      acc = accpool.tile([P, CH], F32, name="acc", tag="acc")
                      for g in range(nkt // GRP):
                          for _ in range(min(per_iter, len(pending))):
                              pending.pop(0)()
                          kts = list(range(GRP * g, GRP * (g + 1)))
                          qk = qkpool.tile([P, GRP * CH], F32, name="qk", tag="qk")
                          if True:
                            for j, kt in enumerate(kts):
                              rt = max(P * kt - CH * ci, 0)
                              if AT != BF16:
                                  rt = min(rt, CH - 256)
                              if "noqk" in variant:
                                  rt = CH - 64
                              nc.tensor.matmul(
                                  qk[:, CH * j + rt:CH * (j + 1)],
                                  kt_sb[h][:, P * kt:P * kt + P],
                                  qt_sb[h][:, CH * ci + rt:CH * ci + CH],
                                  start=True, stop=True)
                          pt = ppool.tile([P, GRP * CH], AT, name="pt", tag="pt")
                          # contiguous exp from the first tile's start col;
                          # cols of tile j+1 below its rt get exp(stale-PSUM)
                          # garbage but are never read by the PV matmul
                          rt0 = max(P * kts[0] - CH * ci, 0)
                          if AT != BF16:
                              rt0 = min(rt0, CH - 256)
                          if "noexp" not in variant:
                            if "esplit" in variant and GRP == 2:
                                # one exp per k-tile: exp(j0) starts while
                                # QK(j1) is still on the PE
                                for j, kt in enumerate(kts):
                                    rtj = max(P * kt - CH * ci, 0)
                                    nc.scalar.activation(
                                        pt[:, CH * j + rtj:CH * (j + 1)],
                                        qk[:, CH * j + rtj:CH * (j + 1)],
                                        Exp, scale=float(SCALE))
                            elif (rt0 >= 2 * P and GRP == 2
                                    and "nosplit" not in variant):
                                # deep-diagonal group: the two computed
                                # regions are small and far apart — exp each
                                # tile separately to skip the stale middle
                                rt1 = rt0 + P
                                nc.scalar.activation(pt[:, rt0:CH],
                                                     qk[:, rt0:CH],
                                                     Exp, scale=float(SCALE))
                                nc.scalar.activation(pt[:, CH + rt1:2 * CH],
                                                     qk[:, CH + rt1:2 * CH],
                                                     Exp, scale=float(SCALE))
                            else:
                                nc.scalar.activation(pt[:, rt0:GRP * CH],
                                                     qk[:, rt0:GRP * CH],
                                                     Exp, scale=float(SCALE))
                          else:
                            w = GRP * CH - rt0
                            src = qk[:, rt0:rt0 + 64].copy()
                            src.ap = src.ap[:1] + [[64, w // 64], [1, 8]]
                            dst = pt[:, rt0:rt0 + 64].copy()
                            dst.ap = dst.ap[:1] + [[64, w // 64], [1, 8]]
                            nc.scalar.activation(dst, src, Exp,
                                                 scale=float(SCALE))
                          if "nomask" not in variant:
                            rts = [P * kt - CH * ci for kt in kts]
                            tmul = (nc.gpsimd.tensor_mul if "gmask" in variant
                                    else nc.vector.tensor_mul)
                            if GRP == 2 and rts[0] >= 0 and "sepmask" not in variant:
                              # both diagonal blocks in one instruction: the
                              # two [128,128] regions sit CH+P=640 cols apart,
                              # so a strided [128, 2, 128] AP covers them; the
                              # mask rides a stride-0 middle dim (same pattern)
                              r3 = pt[:, rts[0]:rts[0] + P].copy()
                              r3.ap = r3.ap[:1] + [[CH + P, 2], [1, P]]
                              m3 = (msk_sb[:, 0:P].unsqueeze(1)
                                    .broadcast_to([P, 2, P]))
                              tmul(r3, r3, m3)
                            else:
                              for j, rt in zip(range(len(kts)), rts):
                                if rt >= 0:
                                  # zero the upper triangle of the 128x128
                                  # diagonal block; cols below rt are skipped
                                  # by the PV matmul
                                  tmul(
                                      pt[:, CH * j + rt:CH * j + rt + P],
                                      pt[:, CH * j + rt:CH * j + rt + P],
                                      msk_sb[:, 0:P])
                          if True:
                            for j, kt in enumerate(kts):
                              rt = P * kt - CH * ci
                              scol = max(rt, 0)
                              if "nopv" in variant:
                                  scol = CH - 64
                              if ("pvsplit" in variant and rt >= 0
                                      and rt + P < CH):
                                  # mask-independent columns first: only the
                                  # 128-wide diagonal block waits for the
                                  # mask multiply, the rest starts right
                                  # after the exp
                                  nc.tensor.matmul(
                                      acc[0:97, rt + P:CH],
                                      v_sb[kt][:, 97 * h:97 * h + 97],
                                      pt[:, CH * j + rt + P:CH * (j + 1)],
                                      start=(kt == 0), stop=False,
                                      skip_group_check=True)
                                  nc.tensor.matmul(
                                      acc[0:97, rt:rt + P],
                                      v_sb[kt][:, 97 * h:97 * h + 97],
                                      pt[:, CH * j + rt:CH * j + rt + P],
                                      start=(kt == 0), stop=(kt == nkt - 1),
                                      skip_group_check=True)
                              else:
                                  nc.tensor.matmul(
                                      acc[0:97, scol:CH],
                                      v_sb[kt][:, 97 * h:97 * h + 97],
                                      pt[:, CH * j + scol:CH * (j + 1)],
                                      start=(kt == 0), stop=(kt == nkt - 1),
                                      skip_group_check=True)
                      # normalize: row 0 of acc is the softmax denominator
                      # (ones column of V' is first). reciprocal reads PSUM
                      # partition 0 directly; normalizing rows 0..96 makes
                      # row 0 = den*(1/den) = 1.0, the ones-row consumed by
                      # the bias row of the output projection.
                      den0 = rpool.tile([1, CH], F32, name="den0", tag="den0")
                      rb = rpool.tile([97, CH], F32, name="rb", tag="rb")
                      on = onpool.tile([97, CH], AT, name=f"on{h}", tag=f"on{h}")
                      if "nonorm" not in variant:
                          nc.vector.reciprocal_approx_fast(out=den0[:], in_=acc[0:1, :])
                          nc.gpsimd.partition_broadcast(rb[:], den0[:], channels=97)
                          nc.vector.tensor_tensor(on[:], acc[0:97, :], rb[:], op=mult)
                      else:
                          nc.vector.reciprocal_approx_fast(out=den0[:], in_=acc[0:1, :])
                          nc.gpsimd.partition_broadcast(rb[:], den0[:], channels=97)
                          o8 = on[0:97, 0:64].copy()
                          o8.ap = o8.ap[:1] + [[64, CH // 64], [1, 8]]
                          a8 = acc[0:97, 0:64].copy()
                          a8.ap = a8.ap[:1] + [[64, CH // 64], [1, 8]]
                          r8 = rb[0:97, 0:64].copy()
                          r8.ap = r8.ap[:1] + [[64, CH // 64], [1, 8]]
                          nc.vector.tensor_tensor(o8, a8, r8, op=mult)
                      on_tiles.append(on)
                  for it in pending:
                      it()
                  pending = []
                  # output projection for this chunk's 4 row-tiles; head 0's
                  # wo rows are [bo ; Wo_0] against on row 0 == 1.0 (bias),
                  # heads 1..3 have a zero row there.
                  for sj in range(4):
                      st = 4 * ci + sj
                      fo = accpool.tile([P, D], F32, name="fo", tag="acc")
                      PJ = 64 if "noproj" in variant else D
                      for h in range(H):
                          nc.tensor.matmul(fo[:, 0:PJ],
                                           on_tiles[h][:, P * sj:P * sj + P],
                                           wo_sb[h][:, 0:PJ], start=(h == 0),
                                           stop=(h == H - 1))
                      fs = onpool.tile([P, D], F32, name="fs", tag="fs", bufs=3)
                      (nc.scalar.copy if ("actcopy" in variant
                                          or "fsact" in variant)
                       else nc.vector.tensor_copy)(fs[:], fo[:])
                      (nc.scalar.dma_start if "odma" in variant
                       else nc.sync.dma_start)(
                          out_d[P * st:P * st + P, :], fs[:])

        pools = [accpool, qkpool, rpool, onpool, ppool, vpool,
                 qkt_pool, xpool, wpool]
        if projpool is not qkpool:
            pools.insert(0, projpool)
        for pool in pools:
            pool.release()

    nc.finalize()
    return nc


_NC_CACHE = None


def get_nc(qbias=False):
    global _NC_CACHE
    if not isinstance(_NC_CACHE, dict):
        _NC_CACHE = {}
    if qbias not in _NC_CACHE:
        _NC_CACHE[qbias] = build_nc(qbias=qbias)
    return _NC_CACHE[qbias]


def host_prep(x, Wq, bq, Wk, bk, Wv, bv, Wo, bo):
    """Build per-core input maps (layout prep only; all FLOPs run on device)."""
    x = np.ascontiguousarray(np.asarray(x, dtype=np.float32))
    Wq = np.ascontiguousarray(np.asarray(Wq, dtype=np.float32))
    Wk = np.ascontiguousarray(np.asarray(Wk, dtype=np.float32))
    Wv = np.ascontiguousarray(np.asarray(Wv, dtype=np.float32))
    Wo = np.ascontiguousarray(np.asarray(Wo, dtype=np.float32))
    bq = np.asarray(bq, dtype=np.float32)
    bv = np.asarray(bv, dtype=np.float32)
    bo = np.asarray(bo, dtype=np.float32)
    # bk is dropped: scores = K^T(Q+bq) + (bk . (Q+bq))(q), and the second
    # term is constant over k at fixed q, so softmax ignores it.

    # V' weights: ones column first, then the 96 head dims
    wvx = np.zeros((D + 1, 97 * H), np.float32)
    for h in range(H):
        wvx[:D, 97 * h + 1:97 * h + 97] = Wv[:, HD * h:HD * h + HD]
        wvx[D, 97 * h + 1:97 * h + 97] = bv[HD * h:HD * h + HD]
        wvx[D, 97 * h] = 1.0

    # output projection rows per head: [bias-or-zero row ; Wo_h]
    woa = np.zeros((97 * H, D), np.float32)
    for h in range(H):
        woa[97 * h + 1:97 * h + 97, :] = Wo[HD * h:HD * h + HD, :]
    woa[0, :] = bo

    # upper-triangle zero mask for the 128x128 diagonal block
    import ml_dtypes
    jj = np.arange(P)[None, :]
    pp = np.arange(P)[:, None]
    mdt = ml_dtypes.bfloat16 if AT_MODE == "bf16" else np.float32
    msk16 = (jj >= pp).astype(mdt)

    bqr = np.ascontiguousarray(bq.reshape(1, D))
    common = dict(wq=Wq, wk=Wk, wvx=wvx, woa=woa, bqr=bqr, msk=msk16,
                  onesrow=np.ones((1, S), np.float32))
    if AT_MODE == "bf16":
        common = {k: (v.astype(mdt) if v.dtype == np.float32 else v)
                  for k, v in common.items()}
        return [dict(xt=np.ascontiguousarray(x[b].T).astype(mdt), **common)
                for b in range(x.shape[0])]
    return [dict(xt=np.ascontiguousarray(x[b].T), **common)
            for b in range(x.shape[0])]


def kernel(**inputs):
    in_maps = host_prep(**inputs)
    need_qbias = bool(np.any(np.asarray(inputs["bq"], dtype=np.float32)))
    nc = get_nc(need_qbias)
    res = run_bass_kernel_spmd(nc, in_maps, core_ids=list(range(N_CORES)))
    return np.stack([res.results[b]["out"] for b in range(N_CORES)], axis=0)


# revision 38
# speedup vs baseline: 1.1155x; 1.1155x over previous
"""Causal multi-head self-attention (B=8, S=2048, D=384, H=4, Hd=96) on 8
Trainium2 NeuronCores.

Sharding: data-parallel over batch — each core processes one batch element,
weights replicated. No collectives needed.

Per-core algorithm (flash-style, fully SBUF-resident, no attention matrix in
HBM); everything feeding the PE is bf16 (1 cycle/col at any moving width,
halves DMA + SBUF so all cross-iteration pools double-buffer):
  - host passes x[b] pre-transposed as xT [384, 2048] in bf16 (layout prep
    and dtype cast only; all FLOPs run on device)
  - QT/KT per head in [96, S] layout (d on partitions), cast bf16 during the
    PSUM->SBUF drain (DVE). K-bias dropped: softmax over k is invariant to
    per-q constants and bk only contributes bk.(Q+bq), constant in k. Q-bias
    rides a rank-1 (bq-row x ones) matmul in the same PSUM accumulation.
  - V' = [ones | V_h] natural layout [S, 97*4] via augmented weights (bias +
    ones column folded into the contraction); ones column FIRST so the
    softmax denominator lands on PSUM partition 0.
  - projections are emitted interleaved with the previous attention chunk
    (own 1-bank PSUM pool) so the scheduler slots their matmuls into
    attention-phase PE gaps and the Activation engine never drains between
    chunks; DMAs split across both HWDGE queues, x first.
  - scoresT[k, q] = KT_h^T @ QT_h per 128-row k-tile and 512-col q-chunk,
    computing only cols >= rt for diagonal tiles; exp on ScalarE
    (PSUM->SBUF bf16, scale=1/sqrt(Hd) folded in, ranges trimmed to the
    computed region); both causal diagonal blocks of a group zeroed by ONE
    strided-AP mask multiply on DVE
  - OT' accumulated in PSUM: row 0 = softmax denominator, rows 1..96 =
    unnormalized head output (d x q)
  - reciprocal (custom DVE approx, reads PSUM row 0 directly) ->
    partition_broadcast (Pool) -> normalize rows 0..96 on DVE; row 0 becomes
    den * (1/den) = 1.0, the ones-row that folds the output bias into head
    0's output-projection matmul (97-partition contraction against
    wo rows [bo ; Wo_h]).
  - output projection summed in PSUM across heads, drained on DVE, DMA out
"""

import os
import sys

sys.path.insert(0, "/opt/trn_rl_repo")

import numpy as np

import concourse.bass as bass
import concourse.tile as tile
from concourse import bacc, mybir
from concourse.bass_utils import run_bass_kernel_spmd

N_CORES = 8
S = 2048
D = 384
H = 4
HD = 96
CH = 512          # q-chunk width (columns per matmul)
NCH = S // CH     # 4 q-chunks
P = 128           # k-tile height / partition dim
KTN = S // P      # 16 k-tiles
SCALE = 1.0 / np.sqrt(HD)

F32 = mybir.dt.float32
BF16 = mybir.dt.bfloat16
MM_DT = os.environ.get("ATTN_MM_DT", "float32r")  # float32r | float32
AT_MODE = os.environ.get("ATTN_AT", "bf16")       # bf16 | f32 attention core


def build_nc(repeat=1, variant=(), loop_n=0, qbias=False):
    """qbias: emit the rank-1 Q-bias matmuls. kernel() passes True only
    when bq is actually nonzero (host_prep sees the values), so the zero-bias
    case skips 16 matmuls without losing generality."""
    nc = bacc.Bacc("TRN2", target_bir_lowering=False, debug=False,
                   enable_asserts=False, num_devices=N_CORES)
    # MF: dtype for tensors feeding matmuls (float32r = single-pass relaxed
    # fp32 on the PE, 4x faster than true fp32; same 4-byte numpy layout).
    # AT: attention-core dtype (bf16: 1 cycle/col at any moving width).
    MF = mybir.dt.float32r if MM_DT == "float32r" else F32
    AT = BF16 if AT_MODE == "bf16" and "at_f32" not in variant else MF

    xt_d = nc.dram_tensor("xt", [D, S], AT, kind="ExternalInput").ap()
    wq_d = nc.dram_tensor("wq", [D, D], AT, kind="ExternalInput").ap()
    wk_d = nc.dram_tensor("wk", [D, D], AT, kind="ExternalInput").ap()
    wvx_d = nc.dram_tensor("wvx", [D + 1, 97 * H], AT, kind="ExternalInput").ap()
    woa_d = nc.dram_tensor("woa", [97 * H, D], AT, kind="ExternalInput").ap()
    bqr_d = (nc.dram_tensor("bqr", [1, D], AT, kind="ExternalInput").ap()
             if qbias else None)
    msk_d = nc.dram_tensor("msk", [P, P], AT, kind="ExternalInput").ap()
    ones_d = nc.dram_tensor("onesrow", [1, S], AT, kind="ExternalInput").ap()
    out_d = nc.dram_tensor("out", [S, D], F32, kind="ExternalOutput").ap()

    Exp = mybir.ActivationFunctionType.Exp
    mult = mybir.AluOpType.mult

    with tile.TileContext(nc) as tc:
        wpool = tc.alloc_tile_pool(name="w", bufs=1)
        xpool = tc.alloc_tile_pool(name="x", bufs=2)
        qkt_pool = tc.alloc_tile_pool(name="qkt", bufs=2)
        vpool = tc.alloc_tile_pool(name="v", bufs=2)
        ppool = tc.alloc_tile_pool(name="p", bufs=(8 if "p8" in variant
                                                    else 4))
        DP3 = (2 if "deep2" in variant else
               (4 if "deep4" in variant else 3))
        onpool = tc.alloc_tile_pool(name="on", bufs=DP3)
        rpool = tc.alloc_tile_pool(name="r", bufs=DP3)
        GRP = 1 if "g1" in variant else 2
        HP = "hp" in variant
        A4 = "a4" in variant
        qkpool = tc.alloc_tile_pool(
            name="qkps", bufs=(4 if GRP == 1 else (3 if HP else 2)),
            space="PSUM")
        accpool = tc.alloc_tile_pool(
            name="accps", bufs=(2 if HP else (4 if A4 else 3)), space="PSUM")
        projpool = ((qkpool if HP or A4 else
                     tc.alloc_tile_pool(name="projps", bufs=1, space="PSUM")))

        import contextlib
        loop_ctx = (tc.For_i(0, loop_n, 1) if loop_n
                    else contextlib.nullcontext())
        with loop_ctx:
          for _rep in range(repeat):
              # ---- load inputs / weights. xt on the SP queue first (it
              # gates the first projections); weights go on the Activation
              # HWDGE queue so both queues fill in parallel.
              wdma = nc.scalar.dma_start
              xt_sb, wq_sb, wk_sb, wv_sb, wo_sb = [], [], [], [], []
              for t in range(3):
                  xt = xpool.tile([P, S], AT, name=f"xt{t}", tag=f"xt{t}")
                  (nc.sync.dma_start if t < 2 else wdma)(
                      xt[:], xt_d[P * t:P * t + P, :])
                  xt_sb.append(xt)
              for t in range(3):
                  wkt = wpool.tile([P, D], AT, name=f"wk{t}", tag=f"wk{t}")
                  wdma(wkt[:], wk_d[P * t:P * t + P, :])
                  wk_sb.append(wkt)
              for t in range(3):
                  wqt = wpool.tile([P, D], AT, name=f"wq{t}", tag=f"wq{t}")
                  wdma(wqt[:], wq_d[P * t:P * t + P, :])
                  wq_sb.append(wqt)
              if qbias:
                  bqr_sb = wpool.tile([1, D], AT, name="bqr", tag="bqr")
                  wdma(bqr_sb[:], bqr_d[:, :])
              msk_sb = wpool.tile([P, P], AT, name="msk", tag="msk")
              wdma(msk_sb[:], msk_d[:, :])
              ones = wpool.tile([1, S], AT, name="ones", tag="ones")
              wdma(ones[:], ones_d[:, :])
              for t in range(3):
                  wvt = wpool.tile([P, 97 * H], AT, name=f"wv{t}", tag=f"wv{t}")
                  wdma(wvt[:], wvx_d[P * t:P * t + P, :])
                  wv_sb.append(wvt)
              wvb = wpool.tile([1, 97 * H], AT, name="wvb", tag="wvb")
              wdma(wvb[:], wvx_d[D:D + 1, :])
              for h in range(H):
                  wot = wpool.tile([97, D], AT, name=f"wo{h}", tag=f"wo{h}")
                  wdma(wot[:], woa_d[97 * h:97 * h + 97, :])
                  wo_sb.append(wot)

              # ---- allocate Q/K/V destination tiles ----
              qt_sb, kt_sb = [], []
              for h in range(H):
                  qt = qkt_pool.tile([HD, S], AT, name=f"qt{h}", tag=f"qt{h}")
                  qt_sb.append(qt)
                  kt = qkt_pool.tile([HD, S], AT, name=f"kt{h}", tag=f"kt{h}")
                  kt_sb.append(kt)
              v_sb = [None] * KTN

              cp = (nc.scalar.copy if "actcopy" in variant
                    else nc.vector.tensor_copy)

              def proj_items(kc, two_pools=False):
                  """Projection work feeding attention chunk kc, as emit
                  thunks: (K,Q) per head first — the next chunk's QK matmuls
                  unblock per head — with V row-tiles interleaved early for
                  the first PV groups. Own 1-bank PSUM pool so these slot
                  into attention-phase PE gaps without blocking score/acc
                  tiles; batch 0 (nothing to overlap with) also borrows the
                  idle score pool for 3-deep pipelining."""
                  seq = []
                  for h in range(H):
                      seq.append(("k", h))
                      seq.append(("q", h))
                      if h < 2:
                          seq.append(("v", 4 * kc + 2 * h))
                          seq.append(("v", 4 * kc + 2 * h + 1))
                  for item_i, (kind, idx) in enumerate(seq):
                      use_qk = HP or A4 or (two_pools and item_i % 3 == 2)
                      def emit(kind=kind, idx=idx, use_qk=use_qk):
                          if use_qk:
                              ps = qkpool.tile([P, GRP * CH], F32, name="pps",
                                               tag="qk")
                          else:
                              ps = projpool.tile([P, CH], F32, name="pps",
                                                 tag="pps")
                          if kind == "v":
                              st = idx
                              for t in range(3):
                                  nc.tensor.matmul(
                                      ps[:, 0:97 * H],
                                      xt_sb[t][:, P * st:P * st + P],
                                      wv_sb[t][:], start=(t == 0), stop=False)
                              nc.tensor.matmul(ps[:, 0:97 * H], ones[:, 0:P],
                                               wvb[:], start=False, stop=True)
                              vt = vpool.tile([P, 97 * H], AT, name=f"v{st}",
                                              tag=f"v{st}")
                              cp(vt[:], ps[:, 0:97 * H])
                              v_sb[st] = vt
                              return
                          h = idx
                          w_sb, dst, has_bias = (
                              (wk_sb, kt_sb, False) if kind == "k"
                              else (wq_sb, qt_sb, qbias))
                          for t in range(3):
                              nc.tensor.matmul(
                                  ps[0:HD, 0:CH],
                                  w_sb[t][:, HD * h:HD * h + HD],
                                  xt_sb[t][:, CH * kc:CH * kc + CH],
                                  start=(t == 0),
                                  stop=(t == 2 and not has_bias))
                          if has_bias:
                              nc.tensor.matmul(
                                  ps[0:HD, 0:CH],
                                  bqr_sb[:, HD * h:HD * h + HD],
                                  ones[:, 0:CH], start=False, stop=True)
                          if "nocopy" in variant:
                              d8 = dst[h][:, CH * kc:CH * kc + 64].copy()
                              d8.ap = d8.ap[:1] + [[64, CH // 64], [1, 8]]
                              s8 = ps[0:HD, 0:64].copy()
                              s8.ap = s8.ap[:1] + [[64, CH // 64], [1, 8]]
                              cp(d8, s8)
                          else:
                              cp(dst[h][:, CH * kc:CH * kc + CH],
                                 ps[0:HD, 0:CH])
                      yield emit

              # ---- attention; chunk ci+1's projections interleave into
              # ---- chunk ci's group loop (one item per group iteration)
              for it in proj_items(0, two_pools=True):
                  it()
              pending = []
              for ci in range(NCH):
                  if ci + 1 < NCH:
                      pending = list(proj_items(ci + 1))
                  n_iters = H * (4 * (ci + 1) // GRP)
                  per_iter = -(-len(pending) // n_iters) if pending else 0
                  on_tiles = []
                  for h in range(H):
                      nkt = 4 * (ci + 1)
                      acc = accpool.tile([P, CH], F32, name="acc", tag="acc")
                      for g in range(nkt // GRP):
                          for _ in range(min(per_iter, len(pending))):
                              pending.pop(0)()
                          kts = list(range(GRP * g, GRP * (g + 1)))
                          qk = qkpool.tile([P, GRP * CH], F32, name="qk", tag="qk")
                          if True:
                            for j, kt in enumerate(kts):
                              rt = max(P * kt - CH * ci, 0)
                              if AT != BF16:
                                  rt = min(rt, CH - 256)
                              if "noqk" in variant:
                                  rt = CH - 64
                              nc.tensor.matmul(
                                  qk[:, CH * j + rt:CH * (j + 1)],
                                  kt_sb[h][:, P * kt:P * kt + P],
                                  qt_sb[h][:, CH * ci + rt:CH * ci + CH],
                                  start=True, stop=True)
                          pt = ppool.tile([P, GRP * CH], AT, name="pt", tag="pt")
                          # contiguous exp from the first tile's start col;
                          # cols of tile j+1 below its rt get exp(stale-PSUM)
                          # garbage but are never read by the PV matmul
                          rt0 = max(P * kts[0] - CH * ci, 0)
                          if AT != BF16:
                              rt0 = min(rt0, CH - 256)
                          if "noexp" not in variant:
                            if "esplit" in variant and GRP == 2:
                                # one exp per k-tile: exp(j0) starts while
                                # QK(j1) is still on the PE
                                for j, kt in enumerate(kts):
                                    rtj = max(P * kt - CH * ci, 0)
                                    nc.scalar.activation(
                                        pt[:, CH * j + rtj:CH * (j + 1)],
                                        qk[:, CH * j + rtj:CH * (j + 1)],
                                        Exp, scale=float(SCALE))
                            elif (rt0 >= 2 * P and GRP == 2
                                    and "nosplit" not in variant):
                                # deep-diagonal group: the two computed
                                # regions are small and far apart — exp each
                                # tile separately to skip the stale middle
                                rt1 = rt0 + P
                                nc.scalar.activation(pt[:, rt0:CH],
                                                     qk[:, rt0:CH],
                                                     Exp, scale=float(SCALE))
                                nc.scalar.activation(pt[:, CH + rt1:2 * CH],
                                                     qk[:, CH + rt1:2 * CH],
                                                     Exp, scale=float(SCALE))
                            else:
                                nc.scalar.activation(pt[:, rt0:GRP * CH],
                                                     qk[:, rt0:GRP * CH],
                                                     Exp, scale=float(SCALE))
                          else:
                            w = GRP * CH - rt0
                            src = qk[:, rt0:rt0 + 64].copy()
                            src.ap = src.ap[:1] + [[64, w // 64], [1, 8]]
                            dst = pt[:, rt0:rt0 + 64].copy()
                            dst.ap = dst.ap[:1] + [[64, w // 64], [1, 8]]
                            nc.scalar.activation(dst, src, Exp,
                                                 scale=float(SCALE))
                          if "nomask" not in variant:
                            rts = [P * kt - CH * ci for kt in kts]
                            tmul = (nc.gpsimd.tensor_mul if "gmask" in variant
                                    else nc.vector.tensor_mul)
                            if GRP == 2 and rts[0] >= 0 and "sepmask" not in variant:
                              # both diagonal blocks in one instruction: the
                              # two [128,128] regions sit CH+P=640 cols apart,
                              # so a strided [128, 2, 128] AP covers them; the
                              # mask rides a stride-0 middle dim (same pattern)
                              r3 = pt[:, rts[0]:rts[0] + P].copy()
                              r3.ap = r3.ap[:1] + [[CH + P, 2], [1, P]]
                              m3 = (msk_sb[:, 0:P].unsqueeze(1)
                                    .broadcast_to([P, 2, P]))
                              tmul(r3, r3, m3)
                            else:
                              for j, rt in zip(range(len(kts)), rts):
                                if rt >= 0:
                                  # zero the upper triangle of the 128x128
                                  # diagonal block; cols below rt are skipped
                                  # by the PV matmul
                                  tmul(
                                      pt[:, CH * j + rt:CH * j + rt + P],
                                      pt[:, CH * j + rt:CH * j + rt + P],
                                      msk_sb[:, 0:P])
                          if True:
                            for j, kt in enumerate(kts):
                              rt = P * kt - CH * ci
                              scol = max(rt, 0)
                              if "nopv" in variant:
                                  scol = CH - 64
                              if ("pvsplit" in variant and rt >= 0
                                      and rt + P < CH):
                                  # mask-independent columns first: only the
                                  # 128-wide diagonal block waits for the
                                  # mask multiply, the rest starts right
                                  # after the exp
                                  nc.tensor.matmul(
                                      acc[0:97, rt + P:CH],
                                      v_sb[kt][:, 97 * h:97 * h + 97],
                                      pt[:, CH * j + rt + P:CH * (j + 1)],
                                      start=(kt == 0), stop=False,
                                      skip_group_check=True)
                                  nc.tensor.matmul(
                                      acc[0:97, rt:rt + P],
                                      v_sb[kt][:, 97 * h:97 * h + 97],
                                      pt[:, CH * j + rt:CH * j + rt + P],
                                      start=(kt == 0), stop=(kt == nkt - 1),
                                      skip_group_check=True)
                              else:
                                  nc.tensor.matmul(
                                      acc[0:97, scol:CH],
                                      v_sb[kt][:, 97 * h:97 * h + 97],
                                      pt[:, CH * j + scol:CH * (j + 1)],
                                      start=(kt == 0), stop=(kt == nkt - 1),
                                      skip_group_check=True)
                      # normalize: row 0 of acc is the softmax denominator
                      # (ones column of V' is first). reciprocal reads PSUM
                      # partition 0 directly; normalizing rows 0..96 makes
                      # row 0 = den*(1/den) = 1.0, the ones-row consumed by
                      # the bias row of the output projection.
                      den0 = rpool.tile([1, CH], F32, name="den0", tag="den0")
                      rb = rpool.tile([97, CH], F32, name="rb", tag="rb")
                      on = onpool.tile([97, CH], AT, name=f"on{h}", tag=f"on{h}")
                      if "nonorm" not in variant:
                          nc.vector.reciprocal_approx_fast(out=den0[:], in_=acc[0:1, :])
                          nc.gpsimd.partition_broadcast(rb[:], den0[:], channels=97)
                          nc.vector.tensor_tensor(on[:], acc[0:97, :], rb[:], op=mult)
                      else:
                          nc.vector.reciprocal_approx_fast(out=den0[:], in_=acc[0:1, :])
                          nc.gpsimd.partition_broadcast(rb[:], den0[:], channels=97)
                          o8 = on[0:97, 0:64].copy()
                          o8.ap = o8.ap[:1] + [[64, CH // 64], [1, 8]]
                          a8 = acc[0:97, 0:64].copy()
                          a8.ap = a8.ap[:1] + [[64, CH // 64], [1, 8]]
                          r8 = rb[0:97, 0:64].copy()
                          r8.ap = r8.ap[:1] + [[64, CH // 64], [1, 8]]
                          nc.vector.tensor_tensor(o8, a8, r8, op=mult)
                      on_tiles.append(on)
                  for it in pending:
                      it()
                  pending = []
                  # output projection for this chunk's 4 row-tiles; head 0's
                  # wo rows are [bo ; Wo_0] against on row 0 == 1.0 (bias),
                  # heads 1..3 have a zero row there.
                  for sj in range(4):
                      st = 4 * ci + sj
                      fo = accpool.tile([P, D], F32, name="fo", tag="acc")
                      PJ = 64 if "noproj" in variant else D
                      for h in range(H):
                          nc.tensor.matmul(fo[:, 0:PJ],
                                           on_tiles[h][:, P * sj:P * sj + P],
                                           wo_sb[h][:, 0:PJ], start=(h == 0),
                                           stop=(h == H - 1))
                      fs = onpool.tile([P, D], F32, name="fs", tag="fs", bufs=3)
                      (nc.scalar.copy if ("actcopy" in variant
                                          or "fsact" in variant)
                       else nc.vector.tensor_copy)(fs[:], fo[:])
                      (nc.scalar.dma_start if "odma" in variant
                       else nc.sync.dma_start)(
                          out_d[P * st:P * st + P, :], fs[:])

        pools = [accpool, qkpool, rpool, onpool, ppool, vpool,
                 qkt_pool, xpool, wpool]
        if projpool is not qkpool:
            pools.insert(0, projpool)
        for pool in pools:
            pool.release()

    nc.finalize()
    return nc


_NC_CACHE = None


def get_nc(qbias=False):
    global _NC_CACHE
    if not isinstance(_NC_CACHE, dict):
        _NC_CACHE = {}
    if qbias not in _NC_CACHE:
        _NC_CACHE[qbias] = build_nc(qbias=qbias)
    return _NC_CACHE[qbias]


def host_prep(x, Wq, bq, Wk, bk, Wv, bv, Wo, bo):
    """Build per-core input maps (layout prep only; all FLOPs run on device)."""
    x = np.ascontiguousarray(np.asarray(x, dtype=np.float32))
    Wq = np.ascontiguousarray(np.asarray(Wq, dtype=np.float32))
    Wk = np.ascontiguousarray(np.asarray(Wk, dtype=np.float32))
    Wv = np.ascontiguousarray(np.asarray(Wv, dtype=np.float32))
    Wo = np.ascontiguousarray(np.asarray(Wo, dtype=np.float32))
    bq = np.asarray(bq, dtype=np.float32)
    bv = np.asarray(bv, dtype=np.float32)
    bo = np.asarray(bo, dtype=np.float32)
    # bk is dropped: scores = K^T(Q+bq) + (bk . (Q+bq))(q), and the second
    # term is constant over k at fixed q, so softmax ignores it.

    # V' weights: ones column first, then the 96 head dims
    wvx = np.zeros((D + 1, 97 * H), np.float32)
    for h in range(H):
        wvx[:D, 97 * h + 1:97 * h + 97] = Wv[:, HD * h:HD * h + HD]
        wvx[D, 97 * h + 1:97 * h + 97] = bv[HD * h:HD * h + HD]
        wvx[D, 97 * h] = 1.0

    # output projection rows per head: [bias-or-zero row ; Wo_h]
    woa = np.zeros((97 * H, D), np.float32)
    for h in range(H):
        woa[97 * h + 1:97 * h + 97, :] = Wo[HD * h:HD * h + HD, :]
    woa[0, :] = bo

    # upper-triangle zero mask for the 128x128 diagonal block
    import ml_dtypes
    jj = np.arange(P)[None, :]
    pp = np.arange(P)[:, None]
    mdt = ml_dtypes.bfloat16 if AT_MODE == "bf16" else np.float32
    msk16 = (jj >= pp).astype(mdt)

    bqr = np.ascontiguousarray(bq.reshape(1, D))
    common = dict(wq=Wq, wk=Wk, wvx=wvx, woa=woa, bqr=bqr, msk=msk16,
                  onesrow=np.ones((1, S), np.float32))
    if AT_MODE == "bf16":
        common = {k: (v.astype(mdt) if v.dtype == np.float32 else v)
                  for k, v in common.items()}
        return [dict(xt=np.ascontiguousarray(x[b].T).astype(mdt), **common)
                for b in range(x.shape[0])]
    return [dict(xt=np.ascontiguousarray(x[b].T), **common)
            for b in range(x.shape[0])]


def kernel(**inputs):
    in_maps = host_prep(**inputs)
    need_qbias = bool(np.any(np.asarray(inputs["bq"], dtype=np.float32)))
    nc = get_nc(need_qbias)
    res = run_bass_kernel_spmd(nc, in_maps, core_ids=list(range(N_CORES)))
    return np.stack([res.results[b]["out"] for b in range(N_CORES)], axis=0)
